# revision 4
# baseline (speedup 1.0000x reference)
"""Trainium2 Bass kernel for nn_CTA_28381143891994.

Continuous-time-attention GRU scan:  per-step ODE (tanh MLP), sigmoid
attention over a fixed query, GRU state update.  Data-parallel over batch
across 8 NeuronCores (8 batch rows per core).

Key reformulations (host-side, exact):
  * score_t = z_t @ (Wk @ q) + bk@q           -- Wk matmul eliminated;
    the (Wk q) column is fused into the ODE-L1 stationary (col 32).
  * c = dt*[ (sum_t a_t z_t) @ Wv + (sum_t a_t) bv ]  -- Wv matmul moved
    out of the scan (rank-1 accumulators r, s)
  * xg_t = x_t @ (Wp @ gru_K) + const         -- proj folded into gru_K
  * z_new = pre + wg*cand with wg = sigmoid(-(xz+hz)),
    pre = zhat - wg*zhat -- shortens the post-sigmoid serial chain.
On-device everything is kept transposed (hidden on partitions, batch on
the free axis) so elementwise work uses all 128 lanes; per-step matmuls
run weights-stationary in fp16 (fp32 PSUM accumulation).  The step is
scheduled so the r-gate PSUM group closes first (its sigmoid gates the
candidate-tanh chain, the longest serial path).
"""

import numpy as np

import concourse.bass as bass
import concourse.mybir as mybir
import concourse.tile as tile
from concourse.bass import ds
from concourse.bass_utils import run_bass_kernel_spmd
from concourse.tile import TileContext, ScopedClock

F16 = mybir.dt.float16
F32 = mybir.dt.float32
AF = mybir.ActivationFunctionType
OP = mybir.AluOpType

B, S, DIN, H = 64, 2048, 512, 512
NCORES = 8
BL = B // NCORES          # 8 batch rows per core
HC = H // 128             # 4 hidden chunks
HB = HC * BL              # 32: one gate third's psum width
G = 3 * H                 # gru gate width 1536
GJ = G // 128             # 12 gate chunks


# ---------------------------------------------------------------------------
# Workaround: this walrus build only accepts a single sync-wait per Drain
# instruction, and the butterfly all-engine barrier emits Drains with
# eq-waits.  Split the tail-drain waits one-per-Drain and use the
# sequencer-level (sem-only) barrier instead.
# ---------------------------------------------------------------------------
def _patched_drain_and_barrier(self, tick_clock, wait_clock):
    nc = self.nc
    d = nc.sync.drain()
    wait_clock.add_sem_waits(d.ins, ScopedClock({None: tick_clock.global_clock}))
    waits = list(d.ins.sync_info.on_wait)
    if len(waits) > 1:
        d.ins.sync_info.on_wait = waits[:1]
        rest = waits[1:]
        while rest:
            d2 = nc.sync.drain()
            d2.ins.sync_info = mybir.SyncInfo(on_wait=rest[:1], on_update=[])
            rest = rest[1:]
    nc.all_engine_barrier(sem_only=True)
    popped = nc._tile_sem_poison_stack.pop()
    assert popped is self._sem_poison
    nc.clear_and_free_semaphores(list(self.sems.allocated().values()))
    nc.all_engine_barrier(sem_only=True)


if not getattr(TileContext, "_cta_drain_patch", False):
    TileContext._drain_and_barrier = _patched_drain_and_barrier
    TileContext._cta_drain_patch = True


def _split_multi_waits(nc):
    """This walrus build encodes at most one sem-wait per instruction.
    Move extra waits onto same-engine NoOps placed just before the owner."""
    ctr = [0]

    def mk_wait_nop(engine, wait):
        ctr[0] += 1
        nop = mybir.InstNoOp(name=f"WSPL-{ctr[0]}", ins=[], outs=[], engine=engine)
        nop.sync_info = mybir.SyncInfo(on_wait=[wait], on_update=[])
        nc.register_instruction(nop, overwrite=True)
        return nop

    for f in nc.m.functions:
        for bb in f.blocks:
            out = []
            changed = False
            for inst in bb.instructions:
                si = inst.sync_info
                if si is not None and si.on_wait and len(si.on_wait) > 1:
                    waits = list(si.on_wait)
                    for w in waits[:-1]:
                        out.append(mk_wait_nop(inst.engine, w))
                    inst.sync_info = mybir.SyncInfo(
                        on_wait=waits[-1:], on_update=list(si.on_update)
                    )
                    changed = True
                out.append(inst)
            if changed:
                bb.instructions = out


# ---------------------------------------------------------------------------
# Program builder
# ---------------------------------------------------------------------------
def build_program(n_steps=S):
    """n_steps: total timesteps incl. t=0 (scan runs t=1..n_steps-1).
    Must be a multiple of 64 and >= 128."""
    assert n_steps % 64 == 0 and n_steps >= 128
    n_bodies = n_steps // 64          # first handled by prologue
    nslots = n_steps // 64            # phase-A 64-step slots

    nc = bass.Bass()
    xt = nc.dram_tensor("xt", [DIN, S, BL], F16, kind="ExternalInput")
    z0t = nc.dram_tensor("z0t", [128, HB], F32, kind="ExternalInput")
    rw = nc.dram_tensor("rw", [128, HC, G], F16, kind="ExternalInput")
    wxg = nc.dram_tensor("wxg", [128, HC, G], F16, kind="ExternalInput")
    wv = nc.dram_tensor("wv", [128, HC, H], F16, kind="ExternalInput")
    w1a = nc.dram_tensor("w1a", [128, HC, 33], F16, kind="ExternalInput")
    w2p = nc.dram_tensor("w2p", [128, 32], F16, kind="ExternalInput")
    w3p = nc.dram_tensor("w3p", [128, H], F16, kind="ExternalInput")
    w3r = nc.dram_tensor("w3r", [128, G], F16, kind="ExternalInput")
    eye = nc.dram_tensor("eye", [128, 128], F16, kind="ExternalInput")
    onesl = nc.dram_tensor("onesl", [128, 128], F16, kind="ExternalInput")
    constj = nc.dram_tensor("constj", [128, GJ], F32, kind="ExternalInput")
    dtcol = nc.dram_tensor("dtcol", [128, 1], F32, kind="ExternalInput")
    sca = nc.dram_tensor("sca", [33, 1], F32, kind="ExternalInput")
    bia = nc.dram_tensor("bia", [33, 1], F32, kind="ExternalInput")
    b1t = nc.dram_tensor("b1t", [32, 1], F32, kind="ExternalInput")
    b2t = nc.dram_tensor("b2t", [32, 1], F32, kind="ExternalInput")
    outt = nc.dram_tensor("outt", [128, HB], F32, kind="ExternalOutput")

    with TileContext(nc) as tc:
        with (
            tc.tile_pool(name="wpool", bufs=1) as wp,
            tc.tile_pool(name="dram", bufs=1, space="DRAM") as dp,
        ):
            xgt = dp.tile([128, n_steps + 64, GJ * BL], F16)

            # ---- persistent SBUF ----
            rw_sb = wp.tile([128, HC, G], F16)
            nc.sync.dma_start(rw_sb[:], rw[:])
            wxg_sb = wp.tile([128, HC, G], F16)
            nc.sync.dma_start(wxg_sb[:], wxg[:])
            wv_sb = wp.tile([128, HC, H], F16)
            nc.sync.dma_start(wv_sb[:], wv[:])
            w1a_sb = wp.tile([128, HC, 33], F16)
            nc.sync.dma_start(w1a_sb[:], w1a[:])
            w2_sb = wp.tile([128, 32], F16)
            nc.sync.dma_start(w2_sb[:], w2p[:])
            w3_sb = wp.tile([128, H], F16)
            nc.sync.dma_start(w3_sb[:], w3p[:])
            w3r_sb = wp.tile([128, G], F16)
            nc.sync.dma_start(w3r_sb[:], w3r[:])
            eye_sb = wp.tile([128, 128], F16)
            nc.sync.dma_start(eye_sb[:], eye[:])
            ones_sb = wp.tile([128, 128], F16)
            nc.sync.dma_start(ones_sb[:], onesl[:])
            constj_sb = wp.tile([128, GJ], F32)
            nc.sync.dma_start(constj_sb[:], constj[:])
            dt_sb = wp.tile([128, 1], F32)
            nc.sync.dma_start(dt_sb[:], dtcol[:])
            sca_sb = wp.tile([33, 1], F32)
            nc.sync.dma_start(sca_sb[:], sca[:])
            bia_sb = wp.tile([33, 1], F32)
            nc.sync.dma_start(bia_sb[:], bia[:])
            b1_sb = wp.tile([32, 1], F32)
            nc.sync.dma_start(b1_sb[:], b1t[:])
            b2_sb = wp.tile([32, 1], F32)
            nc.sync.dma_start(b2_sb[:], b2t[:])

            zA_f = wp.tile([128, HB], F32)
            nc.sync.dma_start(zA_f[:], z0t[:])
            zA_h = wp.tile([128, HB], F16)
            nc.vector.tensor_copy(zA_h[:], zA_f[:])
            zB_f = wp.tile([128, HB], F32)
            zB_h = wp.tile([128, HB], F16)
            zh_f = wp.tile([128, HB], F32)
            rT = wp.tile([128, HB], F32)
            nc.vector.memset(rT[:], 0.0)
            sRow = wp.tile([33, BL], F32)
            nc.vector.memset(sRow[:], 0.0)
            alphaF = wp.tile([33, BL], F32)
            alco = wp.tile([128, BL], F16)
            nc.vector.memset(alco[:], 0.0)
            h1T = wp.tile([128, BL], F16)
            nc.vector.memset(h1T[:], 0.0)
            h2T = wp.tile([128, BL], F16)
            nc.vector.memset(h2T[:], 0.0)
            SB = [wp.tile([128, 16, GJ * BL], F16, name=f"SB{i}")
                  for i in range(4)]
            rgT = wp.tile([128, HB], F32)
            wgT = wp.tile([128, HB], F32)
            t3 = wp.tile([128, HB], F16)
            t4 = wp.tile([128, HB], F16)
            candT = wp.tile([128, HB], F32)
            e2T = wp.tile([128, HB], F32)
            preT = wp.tile([128, HB], F32)
            mT = wp.tile([128, HB], F32)
            rtmp = wp.tile([128, HB], F32)

            # ================= PHASE A: XGT precompute =================
            with (
                tc.tile_pool(name="pa", bufs=3) as pa,
                tc.tile_pool(name="pap", bufs=2, space="PSUM") as pap,
            ):
                for sl in range(nslots):
                    t0 = 64 * sl
                    xts = pa.tile([128, HC, 64, BL], F16, tag="xts")
                    for c in range(HC):
                        nc.sync.dma_start(
                            xts[:, c], xt[c * 128:(c + 1) * 128, t0:t0 + 64, :]
                        )
                    slot = pa.tile([128, 64, GJ, BL], F16, tag="slot")
                    for j in range(GJ):
                        pj = pap.tile([128, 64 * BL], F32, tag="pj")
                        for c in range(HC):
                            nc.tensor.matmul(
                                pj[:],
                                lhsT=wxg_sb[:, c, j * 128:(j + 1) * 128],
                                rhs=xts[:, c],
                                start=(c == 0),
                                stop=(c == HC - 1),
                            )
                        nc.vector.tensor_scalar(
                            slot[:, :, j, :],
                            pj.rearrange("p (t b) -> p t b", b=BL),
                            constj_sb[:, j:j + 1],
                            None,
                            OP.add,
                        )
                    nc.sync.dma_start(
                        xgt[:, t0:t0 + 64, :],
                        slot.rearrange("p t j b -> p t (j b)"),
                    )
                # zero-fill the 64-row lookahead pad past the last real step
                zpad = pa.tile([128, 32, GJ * BL], F16, tag="zpad")
                nc.vector.memset(zpad[:], 0.0)
                nc.sync.dma_start(xgt[:, n_steps:n_steps + 32, :], zpad[:])
                nc.sync.dma_start(xgt[:, n_steps + 32:n_steps + 64, :], zpad[:])

            # ================= PHASE B: the scan =================
            with tc.tile_pool(name="pbp", bufs=1, space="PSUM") as pbp:
                ps_sp = pbp.tile([128, 32], F32)  # W1aug | W2, even/odd halves
                ps_zdot = pbp.tile([128, HB], F32)
                ps_aB = pbp.tile([128, BL], F32)
                ps_z = pbp.tile([128, HB], F32)
                ps_r = pbp.tile([128, HB], F32)
                ps_hh = pbp.tile([128, HB], F32)
                ps_c = pbp.tile([128, HB], F32)
                ps_sB = pbp.tile([128, BL], F32)

                def bs(c):
                    return slice(c * BL, (c + 1) * BL)

                def step(t_par, xgs):
                    # t_par: timestep parity; xgs: [128, GJ*BL] xg row (fp16)
                    z_cur_f, z_cur_h, z_nxt_f, z_nxt_h = (
                        (zA_f, zA_h, zB_f, zB_h) if t_par == 1
                        else (zB_f, zB_h, zA_f, zA_h)
                    )
                    sc1 = slice(0, 8) if t_par == 0 else slice(16, 24)
                    sc2 = slice(8, 16) if t_par == 0 else slice(24, 32)
                    pe = []  # PE instructions in required issue order

                    def pmm(out, lhsT, rhs, start, stop):
                        i = nc.tensor.matmul(out, lhsT=lhsT, rhs=rhs, start=start,
                                             stop=stop, skip_group_check=True)
                        pe.append(i)
                        return i

                    # xg injection: single start=True writer per gate psum
                    pmm(ps_r[:], eye_sb[:], xgs[:, 4 * BL:8 * BL], True, False)
                    pmm(ps_z[:], eye_sb[:], xgs[:, 0:4 * BL], True, False)
                    # ODE layer 1 + score matvec (wkq fused as col 32)
                    for c in range(HC):
                        pmm(ps_sp[:33, sc1], w1a_sb[:, c], z_cur_h[:, bs(c)],
                            c == 0, c == HC - 1)
                    # r-gate recurrent part
                    for j in range(4, 8):
                        for c in range(HC):
                            pmm(ps_r[:, (j - 4) * BL:(j - 3) * BL],
                                rw_sb[:, c, j * 128:(j + 1) * 128],
                                z_cur_h[:, bs(c)], False, False)
                    # ODE layer 2 (h1 ready by now)
                    pmm(ps_sp[:32, sc2], w2_sb[:], h1T[:], True, True)
                    # candidate hh recurrent part
                    for j in range(8, 12):
                        for c in range(HC):
                            pmm(ps_hh[:, (j - 8) * BL:(j - 7) * BL],
                                rw_sb[:, c, j * 128:(j + 1) * 128],
                                z_cur_h[:, bs(c)], c == 0, False)
                    # close r group with the h2 term (dt*W3@R folded host-side)
                    for j in range(4, 8):
                        pmm(ps_r[:, (j - 4) * BL:(j - 3) * BL],
                            w3r_sb[:, j * 128:(j + 1) * 128], h2T[:],
                            False, True)
                    # close hh group
                    for j in range(8, 12):
                        pmm(ps_hh[:, (j - 8) * BL:(j - 7) * BL],
                            w3r_sb[:, j * 128:(j + 1) * 128], h2T[:],
                            False, True)
                    # z-gate recurrent part
                    for j in range(0, 4):
                        for c in range(HC):
                            pmm(ps_z[:, j * BL:(j + 1) * BL],
                                rw_sb[:, c, j * 128:(j + 1) * 128],
                                z_cur_h[:, bs(c)], False, False)
                    for j in range(0, 4):
                        pmm(ps_z[:, j * BL:(j + 1) * BL],
                            w3r_sb[:, j * 128:(j + 1) * 128], h2T[:],
                            False, True)
                    # ODE layer 3 -> zdot psum
                    for c in range(HC):
                        pmm(ps_zdot[:, bs(c)], w3_sb[:, c * 128:(c + 1) * 128],
                            h2T[:], True, True)
                    # alpha broadcast matvec (row 32 of ones is hot)
                    pmm(ps_aB[:], ones_sb[:], alco[:], True, True)

                    # ---- ACT engine, in readiness order ----
                    nc.scalar.activation(h1T[:32, :], ps_sp[:32, sc1], AF.Tanh,
                                         bias=b1_sb[:, 0:1])
                    nc.scalar.activation(h2T[:32, :], ps_sp[:32, sc2], AF.Tanh,
                                         bias=b2_sb[:, 0:1])
                    nc.scalar.activation(rgT[:], ps_r[:], AF.Sigmoid)
                    # wg = 1 - zg = sigmoid(-(xz+hz))
                    nc.scalar.activation(wgT[:], ps_z[:], AF.Sigmoid, scale=-1.0)

                    # ---- DVE critical chain ----
                    nc.vector.tensor_tensor(t3[:], rgT[:], ps_hh[:], OP.mult)
                    nc.vector.tensor_add(t4[:], t3[:], xgs[:, 8 * BL:12 * BL])
                    nc.scalar.activation(candT[:], t4[:], AF.Tanh)
                    # zhat = z + dt*zdot
                    nc.vector.scalar_tensor_tensor(
                        zh_f[:], ps_zdot[:], dt_sb[:, 0:1], z_cur_f[:],
                        OP.mult, OP.add,
                    )
                    # z_new = (zhat - wg*zhat) + wg*cand
                    nc.vector.tensor_tensor(e2T[:], wgT[:], zh_f[:], OP.mult)
                    nc.vector.tensor_tensor(preT[:], zh_f[:], e2T[:], OP.subtract)
                    nc.vector.tensor_tensor(mT[:], wgT[:], candT[:], OP.mult)
                    nc.vector.tensor_add(z_nxt_h[:], preT[:], mT[:])
                    nc.vector.tensor_add(z_nxt_f[:], preT[:], mT[:])

                    # ---- alpha / r / s accumulators (off critical path) ----
                    nc.scalar.activation(alphaF[32:33, :], ps_sp[32:33, sc1],
                                         AF.Sigmoid, bias=bia_sb[32:33, 0:1],
                                         scale=sca_sb[32:33, 0:1])
                    nc.vector.tensor_copy(alco[32:33, :], alphaF[32:33, :])
                    nc.vector.tensor_add(sRow[32:33, :], sRow[32:33, :],
                                         alphaF[32:33, :])
                    nc.vector.tensor_tensor(
                        rtmp.rearrange("p (c b) -> p c b", b=BL),
                        z_cur_f.rearrange("p (c b) -> p c b", b=BL),
                        ps_aB[:, None, :].to_broadcast((128, HC, BL)),
                        OP.mult,
                    )
                    nc.vector.tensor_add(rT[:], rT[:], rtmp[:])

                    # pin the PE issue order (the scheduler otherwise defers
                    # the tanh-chain matmuls behind all the gru pairs)
                    from bass_rust import add_dep_helper
                    for a, b in zip(pe[1:], pe[:-1]):
                        add_dep_helper(a.ins, b.ins, reason="step pe order")

                def load_S(buf, row0):
                    nc.sync.dma_start(buf[:], xgt[:, ds(row0, 16), :])

                def body(iv, first=False):
                    for g in range(4):
                        for k in range(16):
                            t = g * 16 + k
                            if first and t == 0:
                                continue
                            step(t % 2, SB[g][:, k, :])
                        load_S(SB[g], iv * 64 + 64 + g * 16)

                # prologue: steps 1..63 (body 0)
                for g in range(4):
                    load_S(SB[g], g * 16)
                body(0, first=True)
                hint = (mybir.EngineType.PE, mybir.EngineType.DVE,
                        mybir.EngineType.Activation)
                with tc.For_i(1, n_bodies, 1, staggered_reset=True,
                              hint_engines=hint) as iv:
                    body(iv)

                # ================= PHASE C =================
                rh = wp.tile([128, HB], F16)
                nc.vector.tensor_copy(rh[:], rT[:])
                for m in range(HC):
                    for c in range(HC):
                        nc.tensor.matmul(
                            ps_c[:, bs(m)], lhsT=wv_sb[:, c, m * 128:(m + 1) * 128],
                            rhs=rh[:, bs(c)], start=(c == 0), stop=(c == HC - 1),
                        )
                sinv = wp.tile([33, BL], F32)
                nc.vector.reciprocal(sinv[32:33, :], sRow[32:33, :])
                nc.vector.tensor_copy(alco[32:33, :], sinv[32:33, :])
                nc.tensor.matmul(ps_sB[:], lhsT=ones_sb[:], rhs=alco[:],
                                 start=True, stop=True)
                sb_sinv = wp.tile([128, BL], F32)
                nc.vector.tensor_copy(sb_sinv[:], ps_sB[:])
                ct = wp.tile([128, HB], F32)
                nc.vector.tensor_tensor(
                    ct.rearrange("p (c b) -> p c b", b=BL),
                    ps_c.rearrange("p (c b) -> p c b", b=BL),
                    sb_sinv[:, None, :].to_broadcast((128, HC, BL)),
                    OP.mult,
                )
                outT = wp.tile([128, HB], F32)
                nc.vector.tensor_add(outT[:], zB_f[:], ct[:])
                nc.sync.dma_start(outt[:], outT[:])

    import os
    if os.environ.get("CTA_NO_WSPLIT") != "1":
        _split_multi_waits(nc)
    return nc


# ---------------------------------------------------------------------------
# Host side
# ---------------------------------------------------------------------------
_PROGRAM_CACHE = {}


def _get_program(n_steps):
    if n_steps not in _PROGRAM_CACHE:
        _PROGRAM_CACHE[n_steps] = build_program(n_steps)
    return _PROGRAM_CACHE[n_steps]


def _chunked(a):
    """[K*128, N] -> [128, K, N] with chunk index second."""
    k = a.shape[0] // 128
    return np.ascontiguousarray(
        a.reshape(k, 128, *a.shape[1:]).transpose(1, 0, *range(2, a.ndim + 1))
    )


def prepare_host(inputs, n_steps=S):
    inp = {k: np.asarray(v) for k, v in inputs.items()}
    f32 = np.float32
    x = inp["x"].astype(f32, copy=False)
    Wp, bp = inp["Wp"].astype(f32), inp["bp"].astype(f32)
    W1, b1 = inp["ode_W1"].astype(f32), inp["ode_b1"].astype(f32)
    W2, b2 = inp["ode_W2"].astype(f32), inp["ode_b2"].astype(f32)
    W3, b3 = inp["ode_W3"].astype(f32), inp["ode_b3"].astype(f32)
    Wq, bq = inp["Wq"].astype(f32), inp["bq"].astype(f32)
    Wk, bk = inp["Wk"].astype(f32), inp["bk"].astype(f32)
    Wv, bv = inp["Wv"].astype(f32), inp["bv"].astype(f32)
    query = inp["query"].astype(f32)
    tsc = np.float64(inp["time_scale"])
    gK, gR, gb = inp["gru_K"].astype(f32), inp["gru_R"].astype(f32), inp["gru_b"].astype(f32)

    dt = f32(np.log1p(np.exp(tsc)))
    inv = f32(1.0 / np.sqrt(H))
    q = (query @ Wq + bq)[0]
    wkqv = Wk @ q
    bkq = f32(bk @ q)
    Wxg = Wp @ gK
    xg_const = bp @ gK + gb[0]
    constv = xg_const.copy()
    constv[:2 * H] += gb[1][:2 * H]
    assert not np.any(b3), "nonzero ode_b3 path not implemented"
    assert not np.any(gb[1][2 * H:]), "nonzero gru hh-bias path not implemented"

    z0 = x[:, 0] @ Wp + bp  # [B, H] exact fp32

    # W1 (512x32) and the score column Wk@q fused: [128, HC, 33]
    w1a = np.zeros((128, HC, 33), f32)
    w1a[:, :, :32] = _chunked(W1)
    w1a[:, :, 32] = _chunked(wkqv[:, None])[:, :, 0]
    w2p = np.zeros((128, 32), f32); w2p[:32] = W2
    w3p = np.zeros((128, H), f32); w3p[:32] = W3
    w3r = np.zeros((128, G), f32); w3r[:32] = dt * (W3 @ gR)
    onesl = np.zeros((128, 128), f32); onesl[32, :] = 1.0

    shared = {
        "rw": _chunked(gR).astype(np.float16),
        "wxg": _chunked(Wxg).astype(np.float16),
        "wv": _chunked(Wv).astype(np.float16),
        "w1a": w1a.astype(np.float16),
        "w2p": w2p.astype(np.float16),
        "w3p": w3p.astype(np.float16),
        "w3r": w3r.astype(np.float16),
        "eye": np.eye(128, dtype=np.float16),
        "onesl": onesl.astype(np.float16),
        "constj": np.ascontiguousarray(
            constv.reshape(GJ, 128).T).astype(f32),
        "dtcol": np.full((128, 1), dt, f32),
        "sca": np.full((33, 1), inv, f32),
        "bia": np.full((33, 1), bkq * inv, f32),
        "b1t": b1.reshape(32, 1).astype(f32),
        "b2t": b2.reshape(32, 1).astype(f32),
    }

    x16 = x.astype(np.float16)
    in_maps = []
    for ci in range(NCORES):
        xs = x16[ci * BL:(ci + 1) * BL]              # [BL, S, DIN]
        xtc = np.ascontiguousarray(xs.transpose(2, 1, 0))  # [DIN, S, BL]
        z0s = z0[ci * BL:(ci + 1) * BL]              # [BL, H]
        z0tc = np.ascontiguousarray(
            z0s.reshape(BL, HC, 128).transpose(2, 1, 0).reshape(128, HB))
        m = dict(shared)
        m["xt"] = xtc
        m["z0t"] = z0tc.astype(f32)
        in_maps.append(m)
    return in_maps, (Wv, bv)


def assemble_output(results):
    out = np.empty((B, H), np.float32)
    for ci, r in enumerate(results):
        o = r["outt"].reshape(128, HC, BL).transpose(2, 1, 0).reshape(BL, H)
        out[ci * BL:(ci + 1) * BL] = o
    return out


def run(inputs, n_steps=S, **run_kwargs):
    in_maps, _ = prepare_host(inputs, n_steps)
    nc = _get_program(n_steps)
    res = run_bass_kernel_spmd(nc, in_maps, core_ids=list(range(NCORES)),
                               **run_kwargs)
    return assemble_output(res.results), res


def kernel(**inputs):
    out, _ = run(inputs)
    return out


# revision 18
# speedup vs baseline: 1.0954x; 1.0954x over previous
"""Trainium2 Bass kernel for nn_CTA_28381143891994.

Continuous-time-attention GRU scan:  per-step ODE (tanh MLP), sigmoid
attention over a fixed query, GRU state update.  Data-parallel over batch
across 8 NeuronCores (8 batch rows per core).

Key reformulations (host-side, exact):
  * score_t = z_t @ (Wk @ q) + bk@q           -- Wk matmul eliminated;
    the (Wk q) column is fused into the ODE-L1 stationary (col 32).
  * c = dt*[ (sum_t a_t z_t) @ Wv + (sum_t a_t) bv ]  -- Wv matmul moved
    out of the scan (rank-1 accumulators r, s)
  * xg_t = x_t @ (Wp @ gru_K) + const         -- proj folded into gru_K
  * z_new = pre + wg*cand with wg = sigmoid(-(xz+hz)),
    pre = zhat - wg*zhat -- shortens the post-sigmoid serial chain.
On-device everything is kept transposed (hidden on partitions, batch on
the free axis) so elementwise work uses all 128 lanes; per-step matmuls
run weights-stationary in fp16 (fp32 PSUM accumulation).  The step is
scheduled so the r-gate PSUM group closes first (its sigmoid gates the
candidate-tanh chain, the longest serial path).
"""

import numpy as np

import concourse.bass as bass
import concourse.mybir as mybir
import concourse.tile as tile
from concourse.bass import ds
from concourse.bass_utils import run_bass_kernel_spmd
from concourse.tile import TileContext, ScopedClock

F16 = mybir.dt.float16
F32 = mybir.dt.float32
AF = mybir.ActivationFunctionType
OP = mybir.AluOpType

B, S, DIN, H = 64, 2048, 512, 512
NCORES = 8
BL = B // NCORES          # 8 batch rows per core
HC = H // 128             # 4 hidden chunks
HB = HC * BL              # 32: one gate third's psum width
G = 3 * H                 # gru gate width 1536
GJ = G // 128             # 12 gate chunks


# ---------------------------------------------------------------------------
# Workaround: this walrus build only accepts a single sync-wait per Drain
# instruction, and the butterfly all-engine barrier emits Drains with
# eq-waits.  Split the tail-drain waits one-per-Drain and use the
# sequencer-level (sem-only) barrier instead.
# ---------------------------------------------------------------------------
def _patched_drain_and_barrier(self, tick_clock, wait_clock):
    nc = self.nc
    d = nc.sync.drain()
    wait_clock.add_sem_waits(d.ins, ScopedClock({None: tick_clock.global_clock}))
    waits = list(d.ins.sync_info.on_wait)
    if len(waits) > 1:
        d.ins.sync_info.on_wait = waits[:1]
        rest = waits[1:]
        while rest:
            d2 = nc.sync.drain()
            d2.ins.sync_info = mybir.SyncInfo(on_wait=rest[:1], on_update=[])
            rest = rest[1:]
    nc.all_engine_barrier(sem_only=True)
    popped = nc._tile_sem_poison_stack.pop()
    assert popped is self._sem_poison
    nc.clear_and_free_semaphores(list(self.sems.allocated().values()))
    nc.all_engine_barrier(sem_only=True)


if not getattr(TileContext, "_cta_drain_patch", False):
    TileContext._drain_and_barrier = _patched_drain_and_barrier
    TileContext._cta_drain_patch = True


def _split_multi_waits(nc):
    """This walrus build encodes at most one sem-wait per instruction.
    Move extra waits onto same-engine NoOps placed just before the owner."""
    ctr = [0]

    def mk_wait_nop(engine, wait):
        ctr[0] += 1
        nop = mybir.InstNoOp(name=f"WSPL-{ctr[0]}", ins=[], outs=[], engine=engine)
        nop.sync_info = mybir.SyncInfo(on_wait=[wait], on_update=[])
        nc.register_instruction(nop, overwrite=True)
        return nop

    for f in nc.m.functions:
        for bb in f.blocks:
            out = []
            changed = False
            for inst in bb.instructions:
                si = inst.sync_info
                if si is not None and si.on_wait and len(si.on_wait) > 1:
                    waits = list(si.on_wait)
                    for w in waits[:-1]:
                        out.append(mk_wait_nop(inst.engine, w))
                    inst.sync_info = mybir.SyncInfo(
                        on_wait=waits[-1:], on_update=list(si.on_update)
                    )
                    changed = True
                out.append(inst)
            if changed:
                bb.instructions = out


# ---------------------------------------------------------------------------
# Program builder
# ---------------------------------------------------------------------------
def build_program(n_steps=S):
    """n_steps: total timesteps incl. t=0 (scan runs t=1..n_steps-1).
    Must be a multiple of 64 and >= 128."""
    assert n_steps % 64 == 0 and n_steps >= 128
    n_bodies = n_steps // 64          # first handled by prologue
    nslots = n_steps // 64            # phase-A 64-step slots

    nc = bass.Bass()
    xt = nc.dram_tensor("xt", [DIN, S, BL], F16, kind="ExternalInput")
    z0t = nc.dram_tensor("z0t", [128, HB], F32, kind="ExternalInput")
    rw = nc.dram_tensor("rw", [128, HC, G], F16, kind="ExternalInput")
    wxg = nc.dram_tensor("wxg", [128, HC, G], F16, kind="ExternalInput")
    wv = nc.dram_tensor("wv", [128, HC, H], F16, kind="ExternalInput")
    w1a = nc.dram_tensor("w1a", [128, HC, 33], F16, kind="ExternalInput")
    w2p = nc.dram_tensor("w2p", [128, 32], F16, kind="ExternalInput")
    w3p = nc.dram_tensor("w3p", [128, H], F16, kind="ExternalInput")
    w3r = nc.dram_tensor("w3r", [128, G], F16, kind="ExternalInput")
    eye = nc.dram_tensor("eye", [128, 128], F16, kind="ExternalInput")
    onesl = nc.dram_tensor("onesl", [128, 128], F16, kind="ExternalInput")
    constj = nc.dram_tensor("constj", [128, GJ], F32, kind="ExternalInput")
    dtcol = nc.dram_tensor("dtcol", [128, 1], F32, kind="ExternalInput")
    sca = nc.dram_tensor("sca", [33, 1], F32, kind="ExternalInput")
    bia = nc.dram_tensor("bia", [33, 1], F32, kind="ExternalInput")
    b1t = nc.dram_tensor("b1t", [32, 1], F32, kind="ExternalInput")
    b2t = nc.dram_tensor("b2t", [32, 1], F32, kind="ExternalInput")
    outt = nc.dram_tensor("outt", [128, HB], F32, kind="ExternalOutput")
    import os as _os
    _dbg = _os.environ.get("CTA_DBG") == "1"
    if _dbg:
        zdbg1 = nc.dram_tensor("zdbg1", [128, HB], F32, kind="ExternalOutput")
        zdbg2 = nc.dram_tensor("zdbg2", [128, HB], F32, kind="ExternalOutput")
        rdbg = nc.dram_tensor("rdbg", [128, HB], F32, kind="ExternalOutput")
        sdbg = nc.dram_tensor("sdbg", [33, BL], F32, kind="ExternalOutput")
        rgdbg = nc.dram_tensor("rgdbg", [128, HB], F32, kind="ExternalOutput")
        wgdbg = nc.dram_tensor("wgdbg", [128, HB], F32, kind="ExternalOutput")
        cddbg = nc.dram_tensor("cddbg", [128, HB], F32, kind="ExternalOutput")
        zhdbg = nc.dram_tensor("zhdbg", [128, HB], F32, kind="ExternalOutput")
        t3dbg = nc.dram_tensor("t3dbg", [128, HB], F16, kind="ExternalOutput")
        t4dbg = nc.dram_tensor("t4dbg", [128, HB], F16, kind="ExternalOutput")
        hhdbg = nc.dram_tensor("hhdbg", [128, HB], F32, kind="ExternalOutput")
        h1dbg = nc.dram_tensor("h1dbg", [128, BL], F16, kind="ExternalOutput")
        h2dbg = nc.dram_tensor("h2dbg", [128, BL], F16, kind="ExternalOutput")
        xgdbg = nc.dram_tensor("xgdbg", [128, GJ * BL], F16, kind="ExternalOutput")
        spdbg = nc.dram_tensor("spdbg", [128, 32], F32, kind="ExternalOutput")

    with TileContext(nc) as tc:
        with (
            tc.tile_pool(name="wpool", bufs=1) as wp,
            tc.tile_pool(name="dram", bufs=1, space="DRAM") as dp,
        ):
            xgt = dp.tile([128, n_steps + 64, GJ * BL], F16)

            # ---- persistent SBUF ----
            rw_sb = wp.tile([128, HC, G], F16)
            nc.sync.dma_start(rw_sb[:], rw[:])
            wxg_sb = wp.tile([128, HC, G], F16)
            nc.sync.dma_start(wxg_sb[:], wxg[:])
            wv_sb = wp.tile([128, HC, H], F16)
            nc.sync.dma_start(wv_sb[:], wv[:])
            w1a_sb = wp.tile([128, HC, 33], F16)
            nc.sync.dma_start(w1a_sb[:], w1a[:])
            w2_sb = wp.tile([128, 32], F16)
            nc.sync.dma_start(w2_sb[:], w2p[:])
            w3_sb = wp.tile([128, H], F16)
            nc.sync.dma_start(w3_sb[:], w3p[:])
            w3r_sb = wp.tile([128, G], F16)
            nc.sync.dma_start(w3r_sb[:], w3r[:])
            eye_sb = wp.tile([128, 128], F16)
            nc.sync.dma_start(eye_sb[:], eye[:])
            ones_sb = wp.tile([128, 128], F16)
            nc.sync.dma_start(ones_sb[:], onesl[:])
            constj_sb = wp.tile([128, GJ], F32)
            nc.sync.dma_start(constj_sb[:], constj[:])
            dt_sb = wp.tile([128, 1], F32)
            nc.sync.dma_start(dt_sb[:], dtcol[:])
            sca_sb = wp.tile([33, 1], F32)
            nc.sync.dma_start(sca_sb[:], sca[:])
            bia_sb = wp.tile([33, 1], F32)
            nc.sync.dma_start(bia_sb[:], bia[:])
            b1_sb = wp.tile([32, 1], F32)
            nc.sync.dma_start(b1_sb[:], b1t[:])
            b2_sb = wp.tile([32, 1], F32)
            nc.sync.dma_start(b2_sb[:], b2t[:])

            zA_f = wp.tile([128, HB], F32)
            nc.sync.dma_start(zA_f[:], z0t[:])
            zA_h = wp.tile([128, HB], F16)
            nc.vector.tensor_copy(zA_h[:], zA_f[:])
            zB_f = wp.tile([128, HB], F32)
            zB_h = wp.tile([128, HB], F16)
            zh_f = wp.tile([128, HB], F32)
            rT = wp.tile([128, HB], F32)
            nc.vector.memset(rT[:], 0.0)
            sRow = wp.tile([33, BL], F32)
            nc.vector.memset(sRow[:], 0.0)
            alphaF = wp.tile([33, BL], F32)
            alco = wp.tile([128, BL], F16)
            nc.vector.memset(alco[:], 0.0)
            h1T = wp.tile([128, BL], F16)
            nc.vector.memset(h1T[:], 0.0)
            h2T = wp.tile([128, BL], F16)
            nc.vector.memset(h2T[:], 0.0)
            SB = [wp.tile([128, 16, GJ * BL], F16, name=f"SB{i}")
                  for i in range(4)]
            zro16 = wp.tile([128, HB], F16)
            nc.vector.memset(zro16[:], 0.0)
            rgT = wp.tile([128, HB], F32)
            wgT = wp.tile([128, HB], F32)
            t3 = wp.tile([128, HB], F16)
            t4 = wp.tile([128, HB], F16)
            candT = wp.tile([128, HB], F32)
            e2T = wp.tile([128, HB], F32)
            preT = wp.tile([128, HB], F32)
            mT = wp.tile([128, HB], F32)
            rtmp = wp.tile([128, HB], F32)

            # ================= PHASE A: XGT precompute =================
            with (
                tc.tile_pool(name="pa", bufs=3) as pa,
                tc.tile_pool(name="pap", bufs=2, space="PSUM") as pap,
            ):
                for sl in range(nslots):
                    t0 = 64 * sl
                    xts = pa.tile([128, HC, 64, BL], F16, tag="xts")
                    for c in range(HC):
                        nc.sync.dma_start(
                            xts[:, c], xt[c * 128:(c + 1) * 128, t0:t0 + 64, :]
                        )
                    slot = pa.tile([128, 64, GJ, BL], F16, tag="slot")
                    for j in range(GJ):
                        pj = pap.tile([128, 64 * BL], F32, tag="pj")
                        for c in range(HC):
                            nc.tensor.matmul(
                                pj[:],
                                lhsT=wxg_sb[:, c, j * 128:(j + 1) * 128],
                                rhs=xts[:, c],
                                start=(c == 0),
                                stop=(c == HC - 1),
                            )
                        nc.vector.tensor_scalar(
                            slot[:, :, j, :],
                            pj.rearrange("p (t b) -> p t b", b=BL),
                            constj_sb[:, j:j + 1],
                            None,
                            OP.add,
                        )
                    nc.sync.dma_start(
                        xgt[:, t0:t0 + 64, :],
                        slot.rearrange("p t j b -> p t (j b)"),
                    )
                # zero-fill the 64-row lookahead pad past the last real step
                zpad = pa.tile([128, 32, GJ * BL], F16, tag="zpad")
                nc.vector.memset(zpad[:], 0.0)
                nc.sync.dma_start(xgt[:, n_steps:n_steps + 32, :], zpad[:])
                nc.sync.dma_start(xgt[:, n_steps + 32:n_steps + 64, :], zpad[:])

            # ================= PHASE B: the scan =================
            with tc.tile_pool(name="pbp", bufs=1, space="PSUM") as pbp:
                ps_sp = pbp.tile([128, 32], F32)  # W1aug | W2, even/odd halves
                ps_zdot = pbp.tile([128, HB], F32)
                ps_aB = pbp.tile([128, BL], F32)
                ps_z = pbp.tile([128, HB], F32)
                ps_r = pbp.tile([128, HB], F32)
                ps_hh = pbp.tile([128, HB], F32)
                ps_c = pbp.tile([128, HB], F32)
                ps_sB = pbp.tile([128, BL], F32)

                def bs(c):
                    return slice(c * BL, (c + 1) * BL)

                def step(t_par, xgs):
                    # t_par: timestep parity; xgs: [128, GJ*BL] xg row (fp16)
                    z_cur_f, z_cur_h, z_nxt_f, z_nxt_h = (
                        (zA_f, zA_h, zB_f, zB_h) if t_par == 1
                        else (zB_f, zB_h, zA_f, zA_h)
                    )
                    sc1 = slice(0, 8) if t_par == 0 else slice(16, 24)
                    sc2 = slice(8, 16) if t_par == 0 else slice(24, 32)
                    pe = []  # PE instructions in required issue order

                    def pmm(out, lhsT, rhs, start, stop):
                        i = nc.tensor.matmul(out, lhsT=lhsT, rhs=rhs, start=start,
                                             stop=stop, skip_group_check=True)
                        pe.append(i)
                        return i

                    # NOTE: ops are created in dataflow order (Tile deps are
                    # program-order); per-engine issue order is creation order
                    # among that engine's ops.
                    # xg injection: single start=True writer per gate psum
                    pmm(ps_r[:], eye_sb[:], xgs[:, 4 * BL:8 * BL], True, False)
                    pmm(ps_z[:], eye_sb[:], xgs[:, 0:4 * BL], True, False)
                    # single whole-tile start for ps_hh (start wipes the whole
                    # tile's has_written on this hw, so per-region starts with
                    # deferred stops lose earlier regions' accumulation)
                    pmm(ps_hh[:], eye_sb[:], zro16[:], True, False)
                    # ODE layer 1 + score matvec (wkq fused as col 32)
                    for c in range(HC):
                        pmm(ps_sp[:33, sc1], w1a_sb[:, c], z_cur_h[:, bs(c)],
                            c == 0, c == HC - 1)
                    nc.scalar.activation(h1T[:32, :], ps_sp[:32, sc1], AF.Tanh,
                                         bias=b1_sb[:, 0:1])
                    nc.scalar.activation(alphaF[32:33, :], ps_sp[32:33, sc1],
                                         AF.Sigmoid, bias=bia_sb[32:33, 0:1],
                                         scale=sca_sb[32:33, 0:1])
                    nc.vector.tensor_copy(alco[32:33, :], alphaF[32:33, :])
                    nc.vector.tensor_add(sRow[32:33, :], sRow[32:33, :],
                                         alphaF[32:33, :])
                    # r-gate recurrent part
                    for j in range(4, 8):
                        for c in range(HC):
                            pmm(ps_r[:, (j - 4) * BL:(j - 3) * BL],
                                rw_sb[:, c, j * 128:(j + 1) * 128],
                                z_cur_h[:, bs(c)], False, False)
                    # ODE layer 2
                    pmm(ps_sp[:32, sc2], w2_sb[:], h1T[:], True, True)
                    nc.scalar.activation(h2T[:32, :], ps_sp[:32, sc2], AF.Tanh,
                                         bias=b2_sb[:, 0:1])
                    # candidate hh recurrent part
                    for j in range(8, 12):
                        for c in range(HC):
                            pmm(ps_hh[:, (j - 8) * BL:(j - 7) * BL],
                                rw_sb[:, c, j * 128:(j + 1) * 128],
                                z_cur_h[:, bs(c)], False, False)
                    # close r group with the h2 term (dt*W3@R folded host-side)
                    for j in range(4, 8):
                        pmm(ps_r[:, (j - 4) * BL:(j - 3) * BL],
                            w3r_sb[:, j * 128:(j + 1) * 128], h2T[:],
                            False, True)
                    nc.scalar.activation(rgT[:], ps_r[:], AF.Sigmoid)
                    # close hh group
                    for j in range(8, 12):
                        pmm(ps_hh[:, (j - 8) * BL:(j - 7) * BL],
                            w3r_sb[:, j * 128:(j + 1) * 128], h2T[:],
                            False, True)
                    nc.vector.tensor_tensor(t3[:], rgT[:], ps_hh[:], OP.mult)
                    nc.vector.tensor_add(t4[:], t3[:], xgs[:, 8 * BL:12 * BL])
                    # alpha broadcast matvec (row 32 of ones is hot)
                    pmm(ps_aB[:], ones_sb[:], alco[:], True, True)
                    # z-gate recurrent part
                    for j in range(0, 4):
                        for c in range(HC):
                            pmm(ps_z[:, j * BL:(j + 1) * BL],
                                rw_sb[:, c, j * 128:(j + 1) * 128],
                                z_cur_h[:, bs(c)], False, False)
                    for j in range(0, 4):
                        pmm(ps_z[:, j * BL:(j + 1) * BL],
                            w3r_sb[:, j * 128:(j + 1) * 128], h2T[:],
                            False, True)
                    # wg = 1 - zg = sigmoid(-(xz+hz))
                    nc.scalar.activation(wgT[:], ps_z[:], AF.Sigmoid, scale=-1.0)
                    # ODE layer 3 -> zdot psum
                    for c in range(HC):
                        pmm(ps_zdot[:, bs(c)], w3_sb[:, c * 128:(c + 1) * 128],
                            h2T[:], True, True)
                    nc.scalar.activation(candT[:], t4[:], AF.Tanh)
                    # zhat = z + dt*zdot
                    nc.vector.scalar_tensor_tensor(
                        zh_f[:], ps_zdot[:], dt_sb[:, 0:1], z_cur_f[:],
                        OP.mult, OP.add,
                    )
                    # z_new = (zhat - wg*zhat) + wg*cand
                    nc.vector.tensor_tensor(e2T[:], wgT[:], zh_f[:], OP.mult)
                    nc.vector.tensor_tensor(preT[:], zh_f[:], e2T[:], OP.subtract)
                    nc.vector.tensor_tensor(mT[:], wgT[:], candT[:], OP.mult)
                    nc.vector.tensor_add(z_nxt_h[:], preT[:], mT[:])
                    nc.vector.tensor_add(z_nxt_f[:], preT[:], mT[:])
                    # r accumulator (off critical path)
                    nc.vector.tensor_tensor(
                        rtmp.rearrange("p (c b) -> p c b", b=BL),
                        z_cur_f.rearrange("p (c b) -> p c b", b=BL),
                        ps_aB[:, None, :].to_broadcast((128, HC, BL)),
                        OP.mult,
                    )
                    nc.vector.tensor_add(rT[:], rT[:], rtmp[:])

                    # pin the PE issue order (the scheduler otherwise defers
                    # the tanh-chain matmuls behind all the gru pairs)
                    from bass_rust import add_dep_helper
                    for a, b in zip(pe[1:], pe[:-1]):
                        add_dep_helper(a.ins, b.ins, reason="step pe order")

                def load_S(buf, row0):
                    nc.sync.dma_start(buf[:], xgt[:, ds(row0, 16), :])

                def body(iv, first=False):
                    for g in range(4):
                        for k in range(16):
                            t = g * 16 + k
                            if first and t == 0:
                                continue
                            step(t % 2, SB[g][:, k, :])
                        load_S(SB[g], iv * 64 + 64 + g * 16)

                # prologue: steps 1..63 (body 0)
                for g in range(4):
                    load_S(SB[g], g * 16)
                if _dbg:
                    for g in range(4):
                        for k in range(16):
                            t = g * 16 + k
                            if t == 0:
                                continue
                            step(t % 2, SB[g][:, k, :])
                            if t == 1:
                                nc.sync.dma_start(zdbg1[:], zB_f[:])
                                nc.sync.dma_start(rgdbg[:], rgT[:])
                                nc.sync.dma_start(wgdbg[:], wgT[:])
                                nc.sync.dma_start(cddbg[:], candT[:])
                                nc.sync.dma_start(zhdbg[:], zh_f[:])
                                nc.sync.dma_start(t3dbg[:], t3[:])
                                nc.sync.dma_start(t4dbg[:], t4[:])
                                hhcp = wp.tile([128, HB], F32)
                                nc.vector.tensor_copy(hhcp[:], ps_hh[:])
                                nc.sync.dma_start(hhdbg[:], hhcp[:])
                                spcp = wp.tile([128, 32], F32)
                                nc.vector.tensor_copy(spcp[:], ps_sp[:])
                                nc.sync.dma_start(spdbg[:], spcp[:])
                                nc.sync.dma_start(h1dbg[:], h1T[:])
                                nc.sync.dma_start(h2dbg[:], h2T[:])
                                nc.sync.dma_start(xgdbg[:], SB[0][:, 1, :])
                            if t == 2:
                                nc.sync.dma_start(zdbg2[:], zA_f[:])
                        load_S(SB[g], 64 + g * 16)
                else:
                    body(0, first=True)
                hint = (mybir.EngineType.PE, mybir.EngineType.DVE,
                        mybir.EngineType.Activation)
                with tc.For_i(1, n_bodies, 1, staggered_reset=True,
                              hint_engines=hint) as iv:
                    body(iv)

                # ================= PHASE C =================
                if _dbg:
                    nc.sync.dma_start(rdbg[:], rT[:])
                    nc.sync.dma_start(sdbg[:], sRow[:])
                rh = wp.tile([128, HB], F16)
                nc.vector.tensor_copy(rh[:], rT[:])
                for m in range(HC):
                    for c in range(HC):
                        nc.tensor.matmul(
                            ps_c[:, bs(m)], lhsT=wv_sb[:, c, m * 128:(m + 1) * 128],
                            rhs=rh[:, bs(c)], start=(c == 0), stop=(c == HC - 1),
                        )
                sinv = wp.tile([33, BL], F32)
                nc.vector.reciprocal(sinv[32:33, :], sRow[32:33, :])
                nc.vector.tensor_copy(alco[32:33, :], sinv[32:33, :])
                nc.tensor.matmul(ps_sB[:], lhsT=ones_sb[:], rhs=alco[:],
                                 start=True, stop=True)
                sb_sinv = wp.tile([128, BL], F32)
                nc.vector.tensor_copy(sb_sinv[:], ps_sB[:])
                ct = wp.tile([128, HB], F32)
                nc.vector.tensor_tensor(
                    ct.rearrange("p (c b) -> p c b", b=BL),
                    ps_c.rearrange("p (c b) -> p c b", b=BL),
                    sb_sinv[:, None, :].to_broadcast((128, HC, BL)),
                    OP.mult,
                )
                outT = wp.tile([128, HB], F32)
                nc.vector.tensor_add(outT[:], zB_f[:], ct[:])
                nc.sync.dma_start(outt[:], outT[:])

    import os
    if os.environ.get("CTA_NO_WSPLIT") != "1":
        _split_multi_waits(nc)
    return nc


# ---------------------------------------------------------------------------
# Host side
# ---------------------------------------------------------------------------
_PROGRAM_CACHE = {}


def _get_program(n_steps):
    if n_steps not in _PROGRAM_CACHE:
        _PROGRAM_CACHE[n_steps] = build_program(n_steps)
    return _PROGRAM_CACHE[n_steps]


def _chunked(a):
    """[K*128, N] -> [128, K, N] with chunk index second."""
    k = a.shape[0] // 128
    return np.ascontiguousarray(
        a.reshape(k, 128, *a.shape[1:]).transpose(1, 0, *range(2, a.ndim + 1))
    )


def prepare_host(inputs, n_steps=S):
    inp = {k: np.asarray(v) for k, v in inputs.items()}
    f32 = np.float32
    x = inp["x"].astype(f32, copy=False)
    Wp, bp = inp["Wp"].astype(f32), inp["bp"].astype(f32)
    W1, b1 = inp["ode_W1"].astype(f32), inp["ode_b1"].astype(f32)
    W2, b2 = inp["ode_W2"].astype(f32), inp["ode_b2"].astype(f32)
    W3, b3 = inp["ode_W3"].astype(f32), inp["ode_b3"].astype(f32)
    Wq, bq = inp["Wq"].astype(f32), inp["bq"].astype(f32)
    Wk, bk = inp["Wk"].astype(f32), inp["bk"].astype(f32)
    Wv, bv = inp["Wv"].astype(f32), inp["bv"].astype(f32)
    query = inp["query"].astype(f32)
    tsc = np.float64(inp["time_scale"])
    gK, gR, gb = inp["gru_K"].astype(f32), inp["gru_R"].astype(f32), inp["gru_b"].astype(f32)

    dt = f32(np.log1p(np.exp(tsc)))
    inv = f32(1.0 / np.sqrt(H))
    q = (query @ Wq + bq)[0]
    wkqv = Wk @ q
    bkq = f32(bk @ q)
    Wxg = Wp @ gK
    xg_const = bp @ gK + gb[0]
    constv = xg_const.copy()
    constv[:2 * H] += gb[1][:2 * H]
    assert not np.any(b3), "nonzero ode_b3 path not implemented"
    assert not np.any(gb[1][2 * H:]), "nonzero gru hh-bias path not implemented"

    z0 = x[:, 0] @ Wp + bp  # [B, H] exact fp32

    # W1 (512x32) and the score column Wk@q fused: [128, HC, 33]
    w1a = np.zeros((128, HC, 33), f32)
    w1a[:, :, :32] = _chunked(W1)
    w1a[:, :, 32] = _chunked(wkqv[:, None])[:, :, 0]
    w2p = np.zeros((128, 32), f32); w2p[:32] = W2
    w3p = np.zeros((128, H), f32); w3p[:32] = W3
    w3r = np.zeros((128, G), f32); w3r[:32] = dt * (W3 @ gR)
    onesl = np.zeros((128, 128), f32); onesl[32, :] = 1.0

    shared = {
        "rw": _chunked(gR).astype(np.float16),
        "wxg": _chunked(Wxg).astype(np.float16),
        "wv": _chunked(Wv).astype(np.float16),
        "w1a": w1a.astype(np.float16),
        "w2p": w2p.astype(np.float16),
        "w3p": w3p.astype(np.float16),
        "w3r": w3r.astype(np.float16),
        "eye": np.eye(128, dtype=np.float16),
        "onesl": onesl.astype(np.float16),
        "constj": np.ascontiguousarray(
            constv.reshape(GJ, 128).T).astype(f32),
        "dtcol": np.full((128, 1), dt, f32),
        "sca": np.full((33, 1), inv, f32),
        "bia": np.full((33, 1), bkq * inv, f32),
        "b1t": b1.reshape(32, 1).astype(f32),
        "b2t": b2.reshape(32, 1).astype(f32),
    }

    x16 = x.astype(np.float16)
    in_maps = []
    for ci in range(NCORES):
        xs = x16[ci * BL:(ci + 1) * BL]              # [BL, S, DIN]
        xtc = np.ascontiguousarray(xs.transpose(2, 1, 0))  # [DIN, S, BL]
        z0s = z0[ci * BL:(ci + 1) * BL]              # [BL, H]
        z0tc = np.ascontiguousarray(
            z0s.reshape(BL, HC, 128).transpose(2, 1, 0).reshape(128, HB))
        m = dict(shared)
        m["xt"] = xtc
        m["z0t"] = z0tc.astype(f32)
        in_maps.append(m)
    return in_maps, (Wv, bv)


def assemble_output(results):
    out = np.empty((B, H), np.float32)
    for ci, r in enumerate(results):
        o = r["outt"].reshape(128, HC, BL).transpose(2, 1, 0).reshape(BL, H)
        out[ci * BL:(ci + 1) * BL] = o
    return out


def run(inputs, n_steps=S, **run_kwargs):
    in_maps, _ = prepare_host(inputs, n_steps)
    nc = _get_program(n_steps)
    res = run_bass_kernel_spmd(nc, in_maps, core_ids=list(range(NCORES)),
                               **run_kwargs)
    return assemble_output(res.results), res


def kernel(**inputs):
    out, _ = run(inputs)
    return out
